# revision 26
# baseline (speedup 1.0000x reference)
"""AdaProj loss kernel for 8 TRN2 NeuronCores (Bass/Tile), v7.

Math (per reference):
  xn = l2norm(x, 1); Wn = l2norm(W, 2)  [C,S,E]
  q1 = |Wn_c x|^2 ; q2 = (Wn_c x)^T G_c (Wn_c x), G_c = Wn_c Wn_c^T
  logits = q1/sqrt(q2*|x|^2); loss = mean_b( lse_c(s*logits) - s*logits[b,lab] )

v7: host precomputes Wn and the Cholesky G_c = L_c L_c^T, so
  q2 = |M_c x|^2 with M_c = L_c^T Wn_c.  Both quadratic forms become
  squared linear projections of x. Host stacks A = interleave(Wn, M) into
  32 cs-tiles of 128 rows (even tile 2g: Wn rows of classes 4g..4g+3,
  odd tile 2g+1: M rows of the same classes).

Device per tile t: one matmul t_ps = A_t^T x  [128, B] psum; one fused
square-evacuation psum -> fp8 SBUF (Act activation(Square) or DVE
tensor_tensor mult, alternating for engine balance). Pairs (2g, 2g+1)
land in one [128, 2B] fp8 buffer = (ep | up); a single fp8 DoubleRow
indicator matmul per pair accumulates q1 into psum rows 0:64 and q2
into rows 64:128 of the same psum tile.

|x|^2 comes from the host (xss broadcast [64, B] bf16).

The finals (logits -> softmax partials) are software-pipelined ACROSS
iterations: each body ends with qps -> qcopy (persistent SBUF); the
finals chain reading qcopy is emitted interleaved into the NEXT body's
tile loop (and once after the loop for the last iteration), so the
serial logits tail hides under the next iteration's matmul/square work.

Sharding: class-parallel, C=512 -> 64 classes/core. Each core returns
  out[0,:] = sum_{c in shard} exp(s*logits - s) ; out[1,:] = sum_c y*logits
Host: loss = mean( log(sum_i se_i) + s - s*sum_i t0_i ).
"""

import sys

for _p in ("/opt/trn_rl_repo",):
    if _p not in sys.path:
        sys.path.insert(0, _p)

import ml_dtypes
import numpy as np

import bass_rust
import concourse.bass as bass
import concourse.tile as tile
from concourse import mybir
from concourse.bass_utils import run_bass_kernel_spmd

FP32 = mybir.dt.float32
BF16 = mybir.dt.bfloat16
FP8 = mybir.dt.float8e4

B, C, S, E = 1024, 512, 32, 128
NCORES = 8
C_LOC = C // NCORES            # 64 classes per core
NG = C_LOC // 4                # 16 groups of 4 classes
NT = 2 * NG                    # 32 cs-tiles (even: Wn/q1, odd: M/q2)
NB = B // 512                  # psum-bank chunks of the batch

# square-evac mode per tile:
#   'A': Act activation(Square) psum->fp8 (fused evac+square)
#   'V': DVE copy psum->bf16, then DVE TT square ->fp8 (keeps fp8 DR pair)
#   'P': DVE copy psum->bf16, then Pool TT square ->bf16 (single bf16 reduce)
# (DVE cannot square from PSUM: only one PSUM operand per instruction.)
# per-PAIR modes, interleaved so the Act/DVE/Pool square streams overlap
# (a bunched P-stretch serializes on the slow Pool TT)
import os as _os
_PAIRS = (_os.environ.get("V7_PAIRS") or
          "AA,PP,AA,PP,AA,AV,PP,AA,PP,AA,VV,PP,AA,AA,AA,AA").split(",")
MODE = [m for p in _PAIRS for m in p]
assert len(MODE) == NT
# pairs where both tiles produce fp8 use one DoubleRow reduce; others get
# two single-tile reduces through the sliding-window indicator indb
DR_PAIR = [MODE[2 * g] != 'P' and MODE[2 * g + 1] != 'P' for g in range(NG)]

# tile steps at which the 6 pipelined finals ops are emitted
FIN_STEPS = (6, 10, 14, 18, 22, 26)
# defer the from-SBUF squares (V: DVE, P: Pool) this many steps after the
# evacuating copy, so they never delay the PSUM-freeing copies behind them
SQ_LAG = 3
# DMA the output straight from PSUM (no SBUF bounce) — rejected by this
# toolchain (dma_start requires SBUF/DRAM source), keep False
OUT_DMA_PSUM = False


def build_nc(s_val: float, n_iters: int = 1, hw_loop: bool = False) -> bass.Bass:
    nc = bass.Bass()

    a_ext = nc.declare_dram_parameter("aT", [E, NT * 128], BF16, isOutput=False)
    x_ext = nc.declare_dram_parameter("xT", [E, B], BF16, isOutput=False)
    xss_ext = nc.declare_dram_parameter("xss", [C_LOC, B], BF16, isOutput=False)
    yt_ext = nc.declare_dram_parameter("yt", [C_LOC, B], BF16, isOutput=False)
    ind_ext = nc.declare_dram_parameter("indc", [128, NG * 256], FP8, isOutput=False)
    indb_ext = nc.declare_dram_parameter("indb", [128, 252], BF16, isOutput=False)
    out_ext = nc.declare_dram_parameter("out", [128, B], BF16, isOutput=True)

    Mult = mybir.AluOpType.mult
    Exp = mybir.ActivationFunctionType.Exp
    Ln = mybir.ActivationFunctionType.Ln
    Square = mybir.ActivationFunctionType.Square
    DR = mybir.MatmulPerfMode.DoubleRow

    with tile.TileContext(nc) as tc:
        with (
            tc.tile_pool(name="persist", bufs=1) as pp,
            tc.tile_pool(name="xload", bufs=2) as p_x,
            tc.tile_pool(name="aload", bufs=2) as p_a,
            tc.tile_pool(name="sq", bufs=sum(DR_PAIR) + 5) as p_s,
            tc.tile_pool(name="sqs", bufs=2 * (NG - sum(DR_PAIR)) + 6) as p_ss,
            tc.tile_pool(name="cpb", bufs=4) as p_cp,
            tc.tile_pool(name="fin", bufs=2) as p_f,
            tc.tile_pool(name="psT", bufs=3, space="PSUM") as ps_t,
            tc.tile_pool(name="psQ", bufs=1, space="PSUM") as ps_q,
        ):
            # ---- constants, loaded once
            indc = pp.tile([128, NG * 256], FP8, tag="indc")
            nc.sync.dma_start(out=indc[:], in_=ind_ext[:])
            indb = pp.tile([128, 252], BF16, tag="indb")
            nc.sync.dma_start(out=indb[:], in_=indb_ext[:])
            yt = pp.tile([C_LOC, B], BF16, tag="yt")
            nc.sync.dma_start(out=yt[:], in_=yt_ext[:])
            negs = pp.tile([128, 1], FP32, tag="negs")
            nc.vector.memset(negs[:], -s_val)
            # cross-iteration logits-state buffer; memset so the first
            # (pipelined, discarded) finals pass reads finite values
            qcopy = pp.tile([128, B], BF16, tag="qcopy")
            nc.vector.memset(qcopy[:], 1.0)

            def emit_finals_ops(xss):
                """The 6 pipelined finals ops reading qcopy (prev iter).
                Returns (ops, tail) where tail emits the reduce matmuls +
                output DMA (must be emitted after the last main-loop mm)."""
                q2s = p_f.tile([C_LOC, B], BF16, tag="q2s")
                invs = p_f.tile([C_LOC, B], BF16, tag="invs")
                logitsT = p_f.tile([C_LOC, B], BF16, tag="logitsT")
                expz = p_f.tile([C_LOC, B], BF16, tag="expz")
                tl = p_f.tile([C_LOC, B], BF16, tag="tl")

                ops = [
                    # xss lives at partitions 64:128 so both TT operands
                    # share a base partition (walrus same-base rule)
                    lambda: nc.vector.tensor_tensor(
                        out=q2s[:], in0=qcopy[64:128, :], in1=xss[64:128, :],
                        op=Mult
                    ),
                    lambda: nc.scalar.activation(
                        out=invs[:], in_=q2s[:], func=Ln
                    ),
                    lambda: nc.scalar.activation(
                        out=invs[:], in_=invs[:], func=Exp, scale=-0.5
                    ),
                    lambda: nc.vector.tensor_tensor(
                        out=logitsT[:], in0=qcopy[0:64, :], in1=invs[:], op=Mult
                    ),
                    lambda: nc.scalar.activation(
                        out=expz[:], in_=logitsT[:], func=Exp, scale=s_val,
                        bias=negs[0:C_LOC, :],
                    ),
                    lambda: nc.gpsimd.tensor_tensor(
                        out=tl[:], in0=yt[:], in1=logitsT[:], op=Mult
                    ),
                ]

                def tail():
                    # ship the per-class softmax partials; the host sums the
                    # 64 class rows per shard together with the shard-sum
                    nc.scalar.dma_start(out=out_ext[0:C_LOC, :], in_=expz[:])
                    nc.sync.dma_start(out=out_ext[C_LOC:128, :], in_=tl[:])

                return ops, tail

            pipe_state = {}

            def body(pipelined_finals=True):
                prev_sqps = pipe_state.pop("sqps", None)
                # ---- loads (A split in 2 chunks so early tiles start sooner)
                # split the big loads across BOTH HWDGE queues (SP and
                # Act) -- a single queue's bandwidth is the HW bottleneck
                aTb = p_a.tile([128, NT * 128], BF16, tag="aTb")
                half_cols = NT * 64
                nc.sync.dma_start(
                    out=aTb[:, 0:half_cols], in_=a_ext[:, 0:half_cols]
                )
                nc.scalar.dma_start(
                    out=aTb[:, half_cols:], in_=a_ext[:, half_cols:]
                )
                xTb = p_x.tile([128, B], BF16, tag="xTb")
                nc.sync.dma_start(out=xTb[:], in_=x_ext[:])
                xss = p_x.tile([128, B], BF16, tag="xss")
                nc.scalar.dma_start(out=xss[64:128, :], in_=xss_ext[:])

                fin_ops, fin_tail = emit_finals_ops(xss)
                fin_ops = list(fin_ops) if pipelined_finals else []

                qps = (
                    ps_q.tile([128, B], FP32, tag="q", name="qps")
                    if prev_sqps else None
                )
                tpss, sqps = {}, {}

                def emit_mm(t):
                    tps = ps_t.tile([128, B], FP32, tag="t")
                    asl = aTb[:, t * 128 : (t + 1) * 128]
                    for nb in range(NB):
                        nc.tensor.matmul(
                            tps[:, nb * 512 : (nb + 1) * 512], lhsT=asl,
                            rhs=xTb[:, nb * 512 : (nb + 1) * 512],
                            start=True, stop=True,
                        )
                    tpss[t] = tps

                deferred = {}

                def emit_sq(t):
                    g = t // 2
                    if DR_PAIR[g]:
                        if t % 2 == 0:
                            sqps[g] = p_s.tile(
                                [128, 2 * B], FP8, tag="sqp", name="sqp"
                            )
                        dst = sqps[g][:, (t % 2) * B : (t % 2 + 1) * B]
                    else:
                        dst = p_ss.tile([128, B], BF16, tag="sqs", name="sqs")
                        sqps[("s", t)] = dst
                    if MODE[t] == 'A':
                        nc.scalar.activation(
                            out=dst, in_=tpss[t][:], func=Square
                        )
                    else:
                        cpb = p_cp.tile([128, B], BF16, tag="cpb", name="cpb")
                        nc.vector.tensor_copy(cpb[:], tpss[t][:])
                        if MODE[t] == 'V':
                            deferred.setdefault(t + SQ_LAG, []).append(
                                lambda d=dst, c=cpb: nc.vector.tensor_tensor(
                                    out=d, in0=c[:], in1=c[:], op=Mult
                                )
                            )
                        else:
                            deferred.setdefault(t + SQ_LAG, []).append(
                                lambda d=dst, c=cpb: nc.gpsimd.tensor_tensor(
                                    out=d, in0=c[:], in1=c[:], op=Mult
                                )
                            )
                    del tpss[t]

                def _pair_ap(tile2, nb):
                    # [128 part, 2 k-slots, 512 cols] view of [128, 2B]
                    src = tile2[:, nb * 512 : nb * 512 + 512]
                    return bass.AP(
                        tensor=src.tensor, offset=src.offset,
                        ap=[list(src.ap[0]), [B, 2], [1, 512]],
                    )

                def _ind_ap(g):
                    # [128 part, 2 k-slots, 128 out] slice for pair g
                    src = indc[:, 256 * g : 256 * g + 128]
                    return bass.AP(
                        tensor=src.tensor, offset=src.offset,
                        ap=[list(src.ap[0]), [128, 2], [1, 128]],
                    )

                def emit_red(g, srcs, qdst):
                    for nb in range(NB):
                        if DR_PAIR[g]:
                            nc.tensor.matmul(
                                qdst[:, nb * 512 : (nb + 1) * 512],
                                lhsT=_ind_ap(g),
                                rhs=_pair_ap(srcs[g], nb),
                                start=(g == 0), stop=(g == NG - 1),
                                skip_group_check=True, perf_mode=DR,
                            )
                        else:
                            for i in range(2):
                                t = 2 * g + i
                                off = 4 * g + 64 * i
                                nc.tensor.matmul(
                                    qdst[:, nb * 512 : (nb + 1) * 512],
                                    lhsT=indb[:, 124 - off : 252 - off],
                                    rhs=srcs[("s", t)][:, nb * 512 : (nb + 1) * 512],
                                    start=False,
                                    stop=(g == NG - 1 and i == 1),
                                    skip_group_check=True,
                                )

                # ---- main loop: this body's mm+sq stream, interleaved
                # with the PREVIOUS body's reduces (their square buffers are
                # a full body old, so the PE stream never waits on them)
                for step in range(NT + SQ_LAG):
                    if step < NT:
                        emit_mm(step)
                        emit_sq(step)
                    for fn in deferred.pop(step, ()):
                        fn()
                    if fin_ops and step in FIN_STEPS:
                        fin_ops[FIN_STEPS.index(step)]()
                    if prev_sqps and step % 2 == 1 and step < NT:
                        emit_red(step // 2, prev_sqps, qps)

                # previous-previous iteration's softmax partials + output DMA
                if pipelined_finals:
                    fin_tail()

                # stash the previous iteration's q for its finals
                if prev_sqps:
                    nc.vector.tensor_copy(qcopy[:], qps[:])
                pipe_state["sqps"] = sqps

            def flush_finals():
                # drain the cross-body pipeline: finals for the body whose q
                # is already in qcopy, then reduces + finals for the last body
                xss = p_x.tile([128, B], BF16, tag="xss")
                nc.sync.dma_start(out=xss[64:128, :], in_=xss_ext[:])
                ops, tail = emit_finals_ops(xss)
                for op in ops:
                    op()
                tail()
                sqps = pipe_state.pop("sqps", None)
                if sqps:
                    qps = ps_q.tile([128, B], FP32, tag="q")
                    for g in range(NG):
                        # re-use emit_red structure inline
                        for nb in range(NB):
                            if DR_PAIR[g]:
                                nc.tensor.matmul(
                                    qps[:, nb * 512 : (nb + 1) * 512],
                                    lhsT=bass.AP(
                                        tensor=indc.tensor,
                                        offset=indc[:, 256 * g : 256 * g + 128].offset,
                                        ap=[list(indc.ap[0]), [128, 2], [1, 128]],
                                    ),
                                    rhs=bass.AP(
                                        tensor=sqps[g].tensor,
                                        offset=sqps[g][:, nb * 512 : nb * 512 + 512].offset,
                                        ap=[list(sqps[g].ap[0]), [B, 2], [1, 512]],
                                    ),
                                    start=(g == 0), stop=(g == NG - 1),
                                    skip_group_check=True, perf_mode=DR,
                                )
                            else:
                                for i in range(2):
                                    t = 2 * g + i
                                    off = 4 * g + 64 * i
                                    nc.tensor.matmul(
                                        qps[:, nb * 512 : (nb + 1) * 512],
                                        lhsT=indb[:, 124 - off : 252 - off],
                                        rhs=sqps[("s", t)][:, nb * 512 : (nb + 1) * 512],
                                        start=False,
                                        stop=(g == NG - 1 and i == 1),
                                        skip_group_check=True,
                                    )
                    nc.vector.tensor_copy(qcopy[:], qps[:])
                    xss2 = p_x.tile([128, B], BF16, tag="xss")
                    nc.sync.dma_start(out=xss2[64:128, :], in_=xss_ext[:])
                    ops2, tail2 = emit_finals_ops(xss2)
                    for op in ops2:
                        op()
                    tail2()

            if hw_loop:
                with tc.For_i(0, n_iters, 1):
                    body()
                flush_finals()
            else:
                for _ in range(n_iters):
                    body()
                flush_finals()

    # Split multi-wait sync_info into EventSemaphore instructions (HW allows
    # only 1 wait per instruction in this toolchain's walrus).
    bass_rust.move_matmul_waits_to_ldweights(nc.m)
    bass_rust.generate_event_semaphores(nc)
    return nc


def make_aux():
    # indc[k, 256g + 128*ko + m] = 1 iff (ko=0: m = 4g + k//32)
    #                              or  (ko=1: m = 64 + 4g + k//32)
    indc = np.zeros((128, NG * 256), dtype=ml_dtypes.float8_e4m3)
    for g in range(NG):
        for k in range(128):
            c = k // 32
            indc[k, 256 * g + 4 * g + c] = 1.0
            indc[k, 256 * g + 128 + 64 + 4 * g + c] = 1.0
    # indb[k, 124 + k//32] = 1; window [124-off : 252-off] maps partition k
    # to output row off + k//32 (off = 4g + 64*is_u)
    indb = np.zeros((128, 252), dtype=ml_dtypes.bfloat16)
    for k in range(128):
        indb[k, 124 + k // 32] = 1.0
    return indc, indb


def make_in_maps(x, y, W):
    indc, indb = make_aux()
    xT = np.ascontiguousarray(x.T).astype(ml_dtypes.bfloat16)
    xss_row = np.sum(x.astype(np.float64) ** 2, axis=1)
    xss = np.ascontiguousarray(
        np.broadcast_to(xss_row[None, :], (C_LOC, B))
    ).astype(ml_dtypes.bfloat16)

    nrm = np.linalg.norm(W, axis=2, keepdims=True)
    Wn = (W / np.clip(nrm, 1e-12, None)).astype(np.float64)
    G = Wn @ Wn.transpose(0, 2, 1)                    # (C, S, S)
    L = np.linalg.cholesky(G)
    M = (L.transpose(0, 2, 1) @ Wn).astype(np.float32)  # (C, S, E)
    Wn = Wn.astype(np.float32)

    in_maps = []
    for i in range(NCORES):
        c0 = i * C_LOC
        tiles = []
        for g in range(NG):
            cg = c0 + 4 * g
            tiles.append(Wn[cg : cg + 4].reshape(128, E))
            tiles.append(M[cg : cg + 4].reshape(128, E))
        A = np.concatenate(tiles, axis=0)             # (NT*128, E)
        aT = np.ascontiguousarray(A.T).astype(ml_dtypes.bfloat16)
        yt_i = np.ascontiguousarray(
            y[:, c0 : c0 + C_LOC].T
        ).astype(ml_dtypes.bfloat16)
        in_maps.append(
            {
                "aT": aT, "xT": xT, "xss": xss, "yt": yt_i,
                "indc": indc, "indb": indb,
            }
        )
    return in_maps


def combine(outs, s_val):
    se = np.zeros(B, dtype=np.float64)
    t0 = np.zeros(B, dtype=np.float64)
    for o in outs:
        se += o[0:C_LOC].sum(axis=0)
        t0 += o[C_LOC:128].sum(axis=0)
    return np.float32(np.mean(np.log(se) + s_val - s_val * t0))


_CACHE = {}


def kernel(x, y, W, s, **_unused):
    x = np.ascontiguousarray(np.asarray(x, dtype=np.float32))
    y = np.asarray(y, dtype=np.float32)
    W = np.asarray(W, dtype=np.float32)
    s_val = float(np.asarray(s))

    key = ("v7", s_val)
    nc = _CACHE.get(key)
    if nc is None:
        nc = build_nc(s_val)
        _CACHE[key] = nc

    in_maps = make_in_maps(x, y, W)
    res = run_bass_kernel_spmd(nc, in_maps, core_ids=list(range(NCORES)))
    outs = [np.asarray(r["out"], dtype=np.float64) for r in res.results]
    return combine(outs, s_val)


if __name__ == "__main__":
    rng = np.random.default_rng(0)
    x = rng.standard_normal((B, E), dtype=np.float32)
    lab = rng.integers(0, C, size=B)
    y = np.eye(C, dtype=np.float32)[lab]
    W = rng.uniform(-0.1, 0.1, size=(C, S, E)).astype(np.float32)
    s = np.float32(np.sqrt(2.0) * np.log(C - 1.0))
    print(kernel(x=x, y=y, W=W, s=s))


# revision 27
# speedup vs baseline: 1.0410x; 1.0410x over previous
"""AdaProj loss kernel for 8 TRN2 NeuronCores (Bass/Tile), v7.

Math (per reference):
  xn = l2norm(x, 1); Wn = l2norm(W, 2)  [C,S,E]
  q1 = |Wn_c x|^2 ; q2 = (Wn_c x)^T G_c (Wn_c x), G_c = Wn_c Wn_c^T
  logits = q1/sqrt(q2*|x|^2); loss = mean_b( lse_c(s*logits) - s*logits[b,lab] )

v7: host precomputes Wn and the Cholesky G_c = L_c L_c^T, so
  q2 = |M_c x|^2 with M_c = L_c^T Wn_c.  Both quadratic forms become
  squared linear projections of x. Host stacks A = interleave(Wn, M) into
  32 cs-tiles of 128 rows (even tile 2g: Wn rows of classes 4g..4g+3,
  odd tile 2g+1: M rows of the same classes).

Device per tile t: one matmul t_ps = A_t^T x  [128, B] psum; one fused
square-evacuation psum -> fp8 SBUF (Act activation(Square) or DVE
tensor_tensor mult, alternating for engine balance). Pairs (2g, 2g+1)
land in one [128, 2B] fp8 buffer = (ep | up); a single fp8 DoubleRow
indicator matmul per pair accumulates q1 into psum rows 0:64 and q2
into rows 64:128 of the same psum tile.

|x|^2 comes from the host (xss broadcast [64, B] bf16).

The finals (logits -> softmax partials) are software-pipelined ACROSS
iterations: each body ends with qps -> qcopy (persistent SBUF); the
finals chain reading qcopy is emitted interleaved into the NEXT body's
tile loop (and once after the loop for the last iteration), so the
serial logits tail hides under the next iteration's matmul/square work.

Sharding: class-parallel, C=512 -> 64 classes/core. Each core returns
  out[0,:] = sum_{c in shard} exp(s*logits - s) ; out[1,:] = sum_c y*logits
Host: loss = mean( log(sum_i se_i) + s - s*sum_i t0_i ).
"""

import sys

for _p in ("/opt/trn_rl_repo",):
    if _p not in sys.path:
        sys.path.insert(0, _p)

import ml_dtypes
import numpy as np

import bass_rust
import concourse.bass as bass
import concourse.tile as tile
from concourse import mybir
from concourse.bass_utils import run_bass_kernel_spmd

FP32 = mybir.dt.float32
BF16 = mybir.dt.bfloat16
FP8 = mybir.dt.float8e4

B, C, S, E = 1024, 512, 32, 128
NCORES = 8
C_LOC = C // NCORES            # 64 classes per core
NG = C_LOC // 4                # 16 groups of 4 classes
NT = 2 * NG                    # 32 cs-tiles (even: Wn/q1, odd: M/q2)
NB = B // 512                  # psum-bank chunks of the batch

# square-evac mode per tile:
#   'A': Act activation(Square) psum->fp8 (fused evac+square)
#   'V': DVE copy psum->bf16, then DVE TT square ->fp8 (keeps fp8 DR pair)
#   'P': DVE copy psum->bf16, then Pool TT square ->bf16 (single bf16 reduce)
# (DVE cannot square from PSUM: only one PSUM operand per instruction.)
# per-PAIR modes, interleaved so the Act/DVE/Pool square streams overlap
# (a bunched P-stretch serializes on the slow Pool TT)
import os as _os
_PAIRS = (_os.environ.get("V7_PAIRS") or
          "AA,PP,AA,PP,AA,AV,PP,AA,PP,AA,VV,PP,AA,AA,AA,AA").split(",")
MODE = [m for p in _PAIRS for m in p]
assert len(MODE) == NT
# pairs where both tiles produce fp8 use one DoubleRow reduce; others get
# two single-tile reduces through the sliding-window indicator indb
DR_PAIR = [MODE[2 * g] != 'P' and MODE[2 * g + 1] != 'P' for g in range(NG)]

# tile steps at which the 6 pipelined finals ops are emitted
FIN_STEPS = (6, 10, 14, 18, 22, 26)
# defer the from-SBUF squares (V: DVE, P: Pool) this many steps after the
# evacuating copy, so they never delay the PSUM-freeing copies behind them
SQ_LAG = 3
# DMA the output straight from PSUM (no SBUF bounce) — rejected by this
# toolchain (dma_start requires SBUF/DRAM source), keep False
OUT_DMA_PSUM = False


def build_nc(s_val: float, n_iters: int = 1, hw_loop: bool = False) -> bass.Bass:
    nc = bass.Bass()

    a_ext = nc.declare_dram_parameter("aT", [E, NT * 128], BF16, isOutput=False)
    x_ext = nc.declare_dram_parameter("xT", [E, B], BF16, isOutput=False)
    xss_ext = nc.declare_dram_parameter("xss", [C_LOC, B], BF16, isOutput=False)
    yt_ext = nc.declare_dram_parameter("yt", [C_LOC, B], BF16, isOutput=False)
    ind_ext = nc.declare_dram_parameter("indc", [128, NG * 256], FP8, isOutput=False)
    indb_ext = nc.declare_dram_parameter("indb", [128, 252], BF16, isOutput=False)
    out_ext = nc.declare_dram_parameter("out", [128, B], BF16, isOutput=True)

    Mult = mybir.AluOpType.mult
    Exp = mybir.ActivationFunctionType.Exp
    Ln = mybir.ActivationFunctionType.Ln
    Square = mybir.ActivationFunctionType.Square
    DR = mybir.MatmulPerfMode.DoubleRow

    with tile.TileContext(nc) as tc:
        with (
            tc.tile_pool(name="persist", bufs=1) as pp,
            tc.tile_pool(name="xload", bufs=2) as p_x,
            tc.tile_pool(name="aload", bufs=3) as p_a,
            tc.tile_pool(name="sq", bufs=sum(DR_PAIR) + 5) as p_s,
            tc.tile_pool(name="sqs", bufs=2 * (NG - sum(DR_PAIR)) + 6) as p_ss,
            tc.tile_pool(name="cpb", bufs=4) as p_cp,
            tc.tile_pool(name="fin", bufs=2) as p_f,
            tc.tile_pool(name="psT", bufs=3, space="PSUM") as ps_t,
            tc.tile_pool(name="psQ", bufs=1, space="PSUM") as ps_q,
        ):
            # ---- constants, loaded once
            indc = pp.tile([128, NG * 256], FP8, tag="indc")
            nc.sync.dma_start(out=indc[:], in_=ind_ext[:])
            indb = pp.tile([128, 252], BF16, tag="indb")
            nc.sync.dma_start(out=indb[:], in_=indb_ext[:])
            yt = pp.tile([C_LOC, B], BF16, tag="yt")
            nc.sync.dma_start(out=yt[:], in_=yt_ext[:])
            negs = pp.tile([128, 1], FP32, tag="negs")
            nc.vector.memset(negs[:], -s_val)
            # cross-iteration logits-state buffer; memset so the first
            # (pipelined, discarded) finals pass reads finite values
            qcopy = pp.tile([128, B], BF16, tag="qcopy")
            nc.vector.memset(qcopy[:], 1.0)

            def emit_finals_ops(xss):
                """The 6 pipelined finals ops reading qcopy (prev iter).
                Returns (ops, tail) where tail emits the reduce matmuls +
                output DMA (must be emitted after the last main-loop mm)."""
                q2s = p_f.tile([C_LOC, B], BF16, tag="q2s")
                invs = p_f.tile([C_LOC, B], BF16, tag="invs")
                logitsT = p_f.tile([C_LOC, B], BF16, tag="logitsT")
                expz = p_f.tile([C_LOC, B], BF16, tag="expz")
                tl = p_f.tile([C_LOC, B], BF16, tag="tl")

                ops = [
                    # xss lives at partitions 64:128 so both TT operands
                    # share a base partition (walrus same-base rule)
                    lambda: nc.vector.tensor_tensor(
                        out=q2s[:], in0=qcopy[64:128, :], in1=xss[64:128, :],
                        op=Mult
                    ),
                    lambda: nc.scalar.activation(
                        out=invs[:], in_=q2s[:], func=Ln
                    ),
                    lambda: nc.scalar.activation(
                        out=invs[:], in_=invs[:], func=Exp, scale=-0.5
                    ),
                    lambda: nc.vector.tensor_tensor(
                        out=logitsT[:], in0=qcopy[0:64, :], in1=invs[:], op=Mult
                    ),
                    lambda: nc.scalar.activation(
                        out=expz[:], in_=logitsT[:], func=Exp, scale=s_val,
                        bias=negs[0:C_LOC, :],
                    ),
                    lambda: nc.gpsimd.tensor_tensor(
                        out=tl[:], in0=yt[:], in1=logitsT[:], op=Mult
                    ),
                ]

                def tail():
                    # ship the per-class softmax partials; the host sums the
                    # 64 class rows per shard together with the shard-sum
                    nc.sync.dma_start(out=out_ext[0:C_LOC, :], in_=expz[:])
                    nc.sync.dma_start(out=out_ext[C_LOC:128, :], in_=tl[:])

                return ops, tail

            pipe_state = {}

            def body(pipelined_finals=True):
                prev_sqps = pipe_state.pop("sqps", None)
                # ---- loads (A split in 2 chunks so early tiles start sooner)
                aTb = p_a.tile([128, NT * 128], BF16, tag="aTb")
                half_cols = NT * 64
                nc.sync.dma_start(
                    out=aTb[:, 0:half_cols], in_=a_ext[:, 0:half_cols]
                )
                nc.sync.dma_start(
                    out=aTb[:, half_cols:], in_=a_ext[:, half_cols:]
                )
                xTb = p_x.tile([128, B], BF16, tag="xTb")
                nc.sync.dma_start(out=xTb[:], in_=x_ext[:])
                xss = p_x.tile([128, B], BF16, tag="xss")
                nc.sync.dma_start(out=xss[64:128, :], in_=xss_ext[:])

                fin_ops, fin_tail = emit_finals_ops(xss)
                fin_ops = list(fin_ops) if pipelined_finals else []

                qps = (
                    ps_q.tile([128, B], FP32, tag="q", name="qps")
                    if prev_sqps else None
                )
                tpss, sqps = {}, {}

                def emit_mm(t):
                    tps = ps_t.tile([128, B], FP32, tag="t")
                    asl = aTb[:, t * 128 : (t + 1) * 128]
                    for nb in range(NB):
                        nc.tensor.matmul(
                            tps[:, nb * 512 : (nb + 1) * 512], lhsT=asl,
                            rhs=xTb[:, nb * 512 : (nb + 1) * 512],
                            start=True, stop=True,
                        )
                    tpss[t] = tps

                deferred = {}

                def emit_sq(t):
                    g = t // 2
                    if DR_PAIR[g]:
                        if t % 2 == 0:
                            sqps[g] = p_s.tile(
                                [128, 2 * B], FP8, tag="sqp", name="sqp"
                            )
                        dst = sqps[g][:, (t % 2) * B : (t % 2 + 1) * B]
                    else:
                        dst = p_ss.tile([128, B], BF16, tag="sqs", name="sqs")
                        sqps[("s", t)] = dst
                    if MODE[t] == 'A':
                        nc.scalar.activation(
                            out=dst, in_=tpss[t][:], func=Square
                        )
                    else:
                        cpb = p_cp.tile([128, B], BF16, tag="cpb", name="cpb")
                        nc.vector.tensor_copy(cpb[:], tpss[t][:])
                        if MODE[t] == 'V':
                            deferred.setdefault(t + SQ_LAG, []).append(
                                lambda d=dst, c=cpb: nc.vector.tensor_tensor(
                                    out=d, in0=c[:], in1=c[:], op=Mult
                                )
                            )
                        else:
                            deferred.setdefault(t + SQ_LAG, []).append(
                                lambda d=dst, c=cpb: nc.gpsimd.tensor_tensor(
                                    out=d, in0=c[:], in1=c[:], op=Mult
                                )
                            )
                    del tpss[t]

                def _pair_ap(tile2, nb):
                    # [128 part, 2 k-slots, 512 cols] view of [128, 2B]
                    src = tile2[:, nb * 512 : nb * 512 + 512]
                    return bass.AP(
                        tensor=src.tensor, offset=src.offset,
                        ap=[list(src.ap[0]), [B, 2], [1, 512]],
                    )

                def _ind_ap(g):
                    # [128 part, 2 k-slots, 128 out] slice for pair g
                    src = indc[:, 256 * g : 256 * g + 128]
                    return bass.AP(
                        tensor=src.tensor, offset=src.offset,
                        ap=[list(src.ap[0]), [128, 2], [1, 128]],
                    )

                def emit_red(g, srcs, qdst):
                    for nb in range(NB):
                        if DR_PAIR[g]:
                            nc.tensor.matmul(
                                qdst[:, nb * 512 : (nb + 1) * 512],
                                lhsT=_ind_ap(g),
                                rhs=_pair_ap(srcs[g], nb),
                                start=(g == 0), stop=(g == NG - 1),
                                skip_group_check=True, perf_mode=DR,
                            )
                        else:
                            for i in range(2):
                                t = 2 * g + i
                                off = 4 * g + 64 * i
                                nc.tensor.matmul(
                                    qdst[:, nb * 512 : (nb + 1) * 512],
                                    lhsT=indb[:, 124 - off : 252 - off],
                                    rhs=srcs[("s", t)][:, nb * 512 : (nb + 1) * 512],
                                    start=False,
                                    stop=(g == NG - 1 and i == 1),
                                    skip_group_check=True,
                                )

                # ---- main loop: this body's mm+sq stream, interleaved
                # with the PREVIOUS body's reduces (their square buffers are
                # a full body old, so the PE stream never waits on them)
                for step in range(NT + SQ_LAG):
                    if step < NT:
                        emit_mm(step)
                        emit_sq(step)
                    for fn in deferred.pop(step, ()):
                        fn()
                    if fin_ops and step in FIN_STEPS:
                        fin_ops[FIN_STEPS.index(step)]()
                    if prev_sqps and step % 2 == 1 and step < NT:
                        emit_red(step // 2, prev_sqps, qps)

                # previous-previous iteration's softmax partials + output DMA
                if pipelined_finals:
                    fin_tail()

                # stash the previous iteration's q for its finals
                if prev_sqps:
                    nc.vector.tensor_copy(qcopy[:], qps[:])
                pipe_state["sqps"] = sqps

            def flush_finals():
                # drain the cross-body pipeline: finals for the body whose q
                # is already in qcopy, then reduces + finals for the last body
                xss = p_x.tile([128, B], BF16, tag="xss")
                nc.sync.dma_start(out=xss[64:128, :], in_=xss_ext[:])
                ops, tail = emit_finals_ops(xss)
                for op in ops:
                    op()
                tail()
                sqps = pipe_state.pop("sqps", None)
                if sqps:
                    qps = ps_q.tile([128, B], FP32, tag="q")
                    for g in range(NG):
                        # re-use emit_red structure inline
                        for nb in range(NB):
                            if DR_PAIR[g]:
                                nc.tensor.matmul(
                                    qps[:, nb * 512 : (nb + 1) * 512],
                                    lhsT=bass.AP(
                                        tensor=indc.tensor,
                                        offset=indc[:, 256 * g : 256 * g + 128].offset,
                                        ap=[list(indc.ap[0]), [128, 2], [1, 128]],
                                    ),
                                    rhs=bass.AP(
                                        tensor=sqps[g].tensor,
                                        offset=sqps[g][:, nb * 512 : nb * 512 + 512].offset,
                                        ap=[list(sqps[g].ap[0]), [B, 2], [1, 512]],
                                    ),
                                    start=(g == 0), stop=(g == NG - 1),
                                    skip_group_check=True, perf_mode=DR,
                                )
                            else:
                                for i in range(2):
                                    t = 2 * g + i
                                    off = 4 * g + 64 * i
                                    nc.tensor.matmul(
                                        qps[:, nb * 512 : (nb + 1) * 512],
                                        lhsT=indb[:, 124 - off : 252 - off],
                                        rhs=sqps[("s", t)][:, nb * 512 : (nb + 1) * 512],
                                        start=False,
                                        stop=(g == NG - 1 and i == 1),
                                        skip_group_check=True,
                                    )
                    nc.vector.tensor_copy(qcopy[:], qps[:])
                    xss2 = p_x.tile([128, B], BF16, tag="xss")
                    nc.sync.dma_start(out=xss2[64:128, :], in_=xss_ext[:])
                    ops2, tail2 = emit_finals_ops(xss2)
                    for op in ops2:
                        op()
                    tail2()

            if hw_loop:
                with tc.For_i(0, n_iters, 1):
                    body()
                flush_finals()
            else:
                for _ in range(n_iters):
                    body()
                flush_finals()

    # Split multi-wait sync_info into EventSemaphore instructions (HW allows
    # only 1 wait per instruction in this toolchain's walrus).
    bass_rust.move_matmul_waits_to_ldweights(nc.m)
    bass_rust.generate_event_semaphores(nc)
    return nc


def make_aux():
    # indc[k, 256g + 128*ko + m] = 1 iff (ko=0: m = 4g + k//32)
    #                              or  (ko=1: m = 64 + 4g + k//32)
    indc = np.zeros((128, NG * 256), dtype=ml_dtypes.float8_e4m3)
    for g in range(NG):
        for k in range(128):
            c = k // 32
            indc[k, 256 * g + 4 * g + c] = 1.0
            indc[k, 256 * g + 128 + 64 + 4 * g + c] = 1.0
    # indb[k, 124 + k//32] = 1; window [124-off : 252-off] maps partition k
    # to output row off + k//32 (off = 4g + 64*is_u)
    indb = np.zeros((128, 252), dtype=ml_dtypes.bfloat16)
    for k in range(128):
        indb[k, 124 + k // 32] = 1.0
    return indc, indb


def make_in_maps(x, y, W):
    indc, indb = make_aux()
    xT = np.ascontiguousarray(x.T).astype(ml_dtypes.bfloat16)
    xss_row = np.sum(x.astype(np.float64) ** 2, axis=1)
    xss = np.ascontiguousarray(
        np.broadcast_to(xss_row[None, :], (C_LOC, B))
    ).astype(ml_dtypes.bfloat16)

    nrm = np.linalg.norm(W, axis=2, keepdims=True)
    Wn = (W / np.clip(nrm, 1e-12, None)).astype(np.float64)
    G = Wn @ Wn.transpose(0, 2, 1)                    # (C, S, S)
    L = np.linalg.cholesky(G)
    M = (L.transpose(0, 2, 1) @ Wn).astype(np.float32)  # (C, S, E)
    Wn = Wn.astype(np.float32)

    in_maps = []
    for i in range(NCORES):
        c0 = i * C_LOC
        tiles = []
        for g in range(NG):
            cg = c0 + 4 * g
            tiles.append(Wn[cg : cg + 4].reshape(128, E))
            tiles.append(M[cg : cg + 4].reshape(128, E))
        A = np.concatenate(tiles, axis=0)             # (NT*128, E)
        aT = np.ascontiguousarray(A.T).astype(ml_dtypes.bfloat16)
        yt_i = np.ascontiguousarray(
            y[:, c0 : c0 + C_LOC].T
        ).astype(ml_dtypes.bfloat16)
        in_maps.append(
            {
                "aT": aT, "xT": xT, "xss": xss, "yt": yt_i,
                "indc": indc, "indb": indb,
            }
        )
    return in_maps


def combine(outs, s_val):
    se = np.zeros(B, dtype=np.float64)
    t0 = np.zeros(B, dtype=np.float64)
    for o in outs:
        se += o[0:C_LOC].sum(axis=0)
        t0 += o[C_LOC:128].sum(axis=0)
    return np.float32(np.mean(np.log(se) + s_val - s_val * t0))


_CACHE = {}


def kernel(x, y, W, s, **_unused):
    x = np.ascontiguousarray(np.asarray(x, dtype=np.float32))
    y = np.asarray(y, dtype=np.float32)
    W = np.asarray(W, dtype=np.float32)
    s_val = float(np.asarray(s))

    key = ("v7", s_val)
    nc = _CACHE.get(key)
    if nc is None:
        nc = build_nc(s_val)
        _CACHE[key] = nc

    in_maps = make_in_maps(x, y, W)
    res = run_bass_kernel_spmd(nc, in_maps, core_ids=list(range(NCORES)))
    outs = [np.asarray(r["out"], dtype=np.float64) for r in res.results]
    return combine(outs, s_val)


if __name__ == "__main__":
    rng = np.random.default_rng(0)
    x = rng.standard_normal((B, E), dtype=np.float32)
    lab = rng.integers(0, C, size=B)
    y = np.eye(C, dtype=np.float32)[lab]
    W = rng.uniform(-0.1, 0.1, size=(C, S, E)).astype(np.float32)
    s = np.float32(np.sqrt(2.0) * np.log(C - 1.0))
    print(kernel(x=x, y=y, W=W, s=s))
